# revision 1
# baseline (speedup 1.0000x reference)
"""Data-parallel Trainium kernel for nn_ExLayers_61546881352017 (histogram_binning).

Shards the batch dim of `spec` across 8 NeuronCores (pure data parallel, params
replicated), runs the forward pass on each core via jax/XLA-Neuron, and gathers
the full [2048, 256] output.

Self-contained: the forward math is inlined below (mirrors the nn.Module
reference exactly); shapes/sharding are hardcoded for B=2048, L=256, 8 cores.
"""
import math
import functools

import numpy as np
import jax
import jax.numpy as jnp

GATE = 13
NLAYERS = 3
N_CORES = 8


def _cubic_kernel(d):
    a = -0.75
    ad = jnp.abs(d)
    return jnp.where(
        ad <= 1.0,
        ((a + 2.0) * ad - (a + 3.0)) * ad * ad + 1.0,
        jnp.where(ad < 2.0, (((ad - 5.0) * ad + 8.0) * ad - 4.0) * a, 0.0),
    )


def _cubic_interp1d(x, out_size):
    N = x.shape[-1]
    src = jnp.arange(out_size, dtype=x.dtype) * ((N - 1) / (out_size - 1))
    i0 = jnp.floor(src).astype(jnp.int32)
    t = src - i0
    idx = jnp.stack([jnp.clip(i0 + k, 0, N - 1) for k in (-1, 0, 1, 2)], 0)
    w = jnp.stack([_cubic_kernel(t - k) for k in (-1, 0, 1, 2)], 0)
    return (x[..., idx] * w).sum(-2)


def _layer_norm(x, g, b, eps=1e-5):
    mu = x.mean(-1, keepdims=True)
    var = ((x - mu) ** 2).mean(-1, keepdims=True)
    return (x - mu) * jax.lax.rsqrt(var + eps) * g + b


def _encoder_layer(x, wqkv, bqkv, wo, bo, g1, be1, w1, b1, w2, b2, g2, be2):
    d = x.shape[-1]
    qkv = x @ wqkv.T + bqkv
    q, k, v = jnp.split(qkv, 3, axis=-1)
    scores = jnp.einsum("bqd,bkd->bqk", q, k) / math.sqrt(d)
    attn = jax.nn.softmax(scores, axis=-1) @ v
    x = _layer_norm(x + attn @ wo.T + bo, g1, be1)
    ff = jax.nn.relu(x @ w1.T + b1) @ w2.T + b2
    return _layer_norm(x + ff, g2, be2)


def _forward(spec, emb_w, emb_b, wqkv, bqkv, wo, bo, ln1w, ln1b,
             w1, b1, w2, b2, ln2w, ln2b, head_w, head_b, poly_w):
    B, L = spec.shape
    dm = emb_w.shape[0]
    pos = jnp.arange(L, dtype=spec.dtype)[:, None]
    div = jnp.exp(jnp.arange(0, dm, 2, dtype=spec.dtype) * (-math.log(10000.0) / dm))
    pe = jnp.concatenate([jnp.sin(pos * div), jnp.cos(pos * div)], axis=-1)
    x = spec[..., None] * emb_w[:, 0] + emb_b + pe
    for i in range(NLAYERS):
        x = _encoder_layer(x, wqkv[i], bqkv[i], wo[i], bo[i], ln1w[i], ln1b[i],
                           w1[i], b1[i], w2[i], b2[i], ln2w[i], ln2b[i])
    ep = jnp.tanh((x @ head_w.T + head_b)[..., 0])
    ep = ep * (GATE / 2.0) + (GATE / 2.0)
    ep = jnp.clip(ep, 0.0, GATE - 1.0 - 1e-6)
    lower = jnp.floor(ep)
    frac = ep - lower
    il = lower.astype(jnp.int32)
    twohot = (jax.nn.one_hot(il, GATE, dtype=spec.dtype) * (1.0 - frac)[..., None]
              + jax.nn.one_hot(il + 1, GATE, dtype=spec.dtype) * frac[..., None])
    sp = _cubic_interp1d(spec, L + GATE - 1)
    unf = jnp.stack([sp[:, k:k + L] for k in range(GATE)], axis=-1)
    s = (unf * twohot).sum(-1)
    pw = _cubic_interp1d(poly_w, L)
    acc = jnp.zeros_like(s)
    term = jnp.ones_like(s)
    for i in range(pw.shape[0]):
        acc = acc + term * pw[i]
        term = term * s
    return s + acc


@functools.partial(jax.pmap, axis_name="b",
                   in_axes=(0,) + (None,) * 17, devices=jax.devices()[:N_CORES])
def _forward_pmap(spec, emb_w, emb_b, wqkv, bqkv, wo, bo, ln1w, ln1b,
                  w1, b1, w2, b2, ln2w, ln2b, head_w, head_b, poly_w):
    return _forward(spec, emb_w, emb_b, wqkv, bqkv, wo, bo, ln1w, ln1b,
                    w1, b1, w2, b2, ln2w, ln2b, head_w, head_b, poly_w)


def kernel(**inputs) -> np.ndarray:
    spec = np.asarray(inputs["spec"], dtype=np.float32)
    B, L = spec.shape
    assert B % N_CORES == 0, (B, N_CORES)
    spec_sharded = spec.reshape(N_CORES, B // N_CORES, L)
    params = [
        np.asarray(inputs[k], dtype=np.float32)
        for k in ("emb_w", "emb_b", "wqkv", "bqkv", "wo", "bo", "ln1w", "ln1b",
                  "w1", "b1", "w2", "b2", "ln2w", "ln2b", "head_w", "head_b",
                  "poly_w")
    ]
    out = _forward_pmap(spec_sharded, *params)
    return np.asarray(out).reshape(B, L).astype(np.float32)
